# revision 27
# baseline (speedup 1.0000x reference)
"""Additive (Bahdanau) attention on 8 Trainium2 NeuronCores.

Reference math (BS=2, J=512, T=256, D=512):
    kk = k @ Wk.T                  [b, J, D]
    qq = q @ Wq.T + bq             [b, T, D]
    scores[b,j,t] = sum_d we[d] * tanh(kk[b,j,d] + qq[b,t,d])
    scores masked to -1e9 where mask[b,j,0]==0
    alphas = softmax_j(scores^T)   [b, T, J]
    context = alphas @ v           [b, T, D]
    returns (context, alphas)

Sharding: the 512 (b, t) query rows are split into 8 blocks of 64 (cores 0-3
take b=0, cores 4-7 take b=1); softmax over j is independent per row.

Sparsity: masked j rows produce exactly-zero alphas (exp(-1e9-max) underflows),
so the host compacts k/v to the unmasked j set before launch and scatters
alphas back afterwards.  This halves the dominant tanh work.

Device pipeline per core (jp = padded compact J, bf16 energy path):
    0. Inputs arrive as host-prebuilt SBUF images (exact on-chip layout) in
       three large contiguous DMAs, ordered so projections start early.
    1. PE: kkT[e, j] and qqT[e, t] projections in bf16 (single-pass matmuls),
       bq added via a rank-1 matmul; both evacuated to SBUF.
    2. Per group of TGRP queries: DVE + GpSimd tensor_scalar_add broadcast
       qq[:, t] onto kkT building S supertiles (the two engines run
       concurrently since the scalar-AP op keeps DVE at 1x = dedicated SBUF
       ports); ACT runs one unbiased in-place tanh per (chunk, group); PE
       reduces over e with a `we` sliding-window stationary (bf16, FWL) that
       lands each t's scores in its own PSUM row of one long accumulation
       group.  A rank-1 fp32 matmul adds -1e9 to pad columns.
    3. Row softmax in fp32: DVE -max, ACT exp (bias=-max, fused row-sum),
       DVE reciprocal + scale.
    4. PE transposes alphas (identity matmul), fp32 context matmul, DMA out.
"""

import sys

sys.path.insert(0, "/opt/trn_rl_repo")

import numpy as np
from contextlib import ExitStack

import concourse.bass as bass
import concourse.bacc as bacc
import concourse.tile as tile
from concourse import mybir
from concourse.bass_utils import run_bass_kernel_spmd

BS, J, T, D = 2, 512, 256, 512
NCORES = 8
TBLK = BS * T // NCORES  # 64 query rows per core
EC = D // 128            # 4 feature chunks
TGRP = 16                # queries per tanh supertile
F32 = mybir.dt.float32
BF16 = mybir.dt.bfloat16
NPBF16 = mybir.dt.np(BF16)
AF = mybir.ActivationFunctionType

_BUILD_CACHE: dict[int, bass.Bass] = {}


def _layout(jp: int):
    """Column offsets inside the two SBUF input images."""
    nch = (jp + 127) // 128
    bf_a = {"Wk": 0, "kT": EC * D}
    fa = EC * D + EC * jp  # bf16 image A total
    bf_b = {"Wq": 0, "qT": EC * D, "wew": EC * D + EC * TBLK,
            "bq": EC * D + EC * TBLK + EC * 2 * 256}
    fb = bf_b["bq"] + D
    f32 = {"v": 0, "mrow": nch * D, "iden": nch * D + jp}
    ff = f32["iden"] + TBLK
    return nch, bf_a, fa, bf_b, fb, f32, ff


def build_nc(jp: int) -> bass.Bass:
    """Build the single-core Bass program (SPMD across all 8 cores)."""
    nc = bacc.Bacc("TRN2", target_bir_lowering=False, debug=True)
    nch, bf_a, fa, bf_b, fb, f32o, ff = _layout(jp)

    imgA = nc.dram_tensor("imgA", [128, fa], BF16, kind="ExternalInput")
    imgB = nc.dram_tensor("imgB", [128, fb], BF16, kind="ExternalInput")
    imgF = nc.dram_tensor("imgF", [128, ff], F32, kind="ExternalInput")
    ctx_out = nc.dram_tensor("ctx_out", [TBLK, D], F32, kind="ExternalOutput")
    alp_out = nc.dram_tensor("alp_out", [TBLK, jp], F32, kind="ExternalOutput")

    jch = [(i * 128, min(128, jp - i * 128)) for i in range(nch)]

    with tile.TileContext(nc) as tc, ExitStack() as ctx:
        const = ctx.enter_context(tc.tile_pool(name="const", bufs=1))
        work = ctx.enter_context(tc.tile_pool(name="work", bufs=2))
        spool = ctx.enter_context(tc.tile_pool(name="spool", bufs=3))
        pkk = ctx.enter_context(tc.tile_pool(name="pkk", bufs=4, space="PSUM"))
        pqq = ctx.enter_context(tc.tile_pool(name="pqq", bufs=1, space="PSUM"))
        psc = ctx.enter_context(tc.tile_pool(name="psc", bufs=2, space="PSUM"))

        # ---------------- loads: 3 image DMAs ----------------
        sbA = const.tile([128, fa], BF16, tag="imgA")
        nc.sync.dma_start(out=sbA, in_=imgA[:, :])
        sbB = const.tile([128, fb], BF16, tag="imgB")
        nc.sync.dma_start(out=sbB, in_=imgB[:, :])
        sbF = const.tile([128, ff], F32, tag="imgF")
        nc.sync.dma_start(out=sbF, in_=imgF[:, :])

        def sl(img, off, n, pat=None, p0=128, **kw):
            ap = img[0:p0, off : off + n]
            return ap.rearrange(pat, **kw) if pat else ap

        sb_Wk = sl(sbA, bf_a["Wk"], EC * D, "p (c e) -> p c e", c=EC)
        sb_kT = sl(sbA, bf_a["kT"], EC * jp, "p (c j) -> p c j", c=EC)
        sb_Wq = sl(sbB, bf_b["Wq"], EC * D, "p (c e) -> p c e", c=EC)
        sb_qT = sl(sbB, bf_b["qT"], EC * TBLK, "p (c t) -> p c t", c=EC)
        sb_wew = sl(sbB, bf_b["wew"], EC * 2 * 256, "p (c r w) -> p c r w", c=EC, r=2)
        sb_bq = sl(sbB, bf_b["bq"], D, p0=1)
        sb_v = [sl(sbF, f32o["v"] + i * D, D, p0=jw) for i, (j0, jw) in enumerate(jch)]
        sb_mrow = sl(sbF, f32o["mrow"], jp, p0=1)
        sb_id = sl(sbF, f32o["iden"], TBLK, p0=TBLK)

        on1 = const.tile([1, 128], F32, tag="on1")
        nc.vector.memset(on1, 1.0)
        on64 = const.tile([1, TBLK], BF16, tag="on64")
        nc.vector.memset(on64, 1.0)

        # ---------------- projections (bf16 in, fp32 PSUM, bf16 out) -------
        kk_sb = const.tile([128, EC, jp], BF16, tag="kksb")
        for e in range(EC):
            kt = pkk.tile([128, jp], F32, tag="kk")
            for c in range(EC):
                nc.tensor.matmul(
                    out=kt,
                    lhsT=sb_Wk[:, c, e * 128 : (e + 1) * 128],
                    rhs=sb_kT[:, c, :],
                    start=(c == 0),
                    stop=(c == EC - 1),
                )
            nc.scalar.copy(kk_sb[:, e, :], kt)
        qq_sb = const.tile([128, EC, TBLK], F32, tag="qqsb")
        for e in range(EC):
            qps = pqq.tile([128, TBLK], F32, tag="qq")
            for c in range(EC):
                nc.tensor.matmul(
                    out=qps,
                    lhsT=sb_Wq[:, c, e * 128 : (e + 1) * 128],
                    rhs=sb_qT[:, c, :],
                    start=(c == 0),
                    stop=False,
                )
            nc.tensor.matmul(
                out=qps,
                lhsT=sb_bq[0:1, e * 128 : (e + 1) * 128],
                rhs=on64,
                start=False,
                stop=True,
            )
            nc.vector.tensor_copy(qq_sb[:, e, :], qps)

        # ---------------- energy + scores ----------------
        ngrp = TBLK // TGRP
        scores_ps = psc.tile([128, jp], F32, tag="scores")
        # pad columns get -1e9 in every row: rank-1 ones^T x mrow
        nc.tensor.matmul(out=scores_ps, lhsT=on1, rhs=sb_mrow, start=True, stop=False)
        # (chunk, i) slots whose add+tanh runs fused on ACT: run-edge slices
        # only, so the remaining tanh supertiles stay contiguous.  These
        # rebalance work from the 1x-capped DVE scalar-add onto ACT.
        fused = {(c, 0) for c in range(EC) if c < 3}
        # Small first group lets the PE start early; small last group keeps
        # the final tanh burst off the softmax critical path.
        sizes = [8, 16, 16, 16, 8]
        assert sum(sizes) == TBLK
        t_base = 0
        for g, gn in enumerate(sizes):
            sts = [spool.tile([128, TGRP, jp], BF16, tag=f"S{c}", name=f"S{c}")
                   for c in range(EC)]
            for c in range(EC):
                for i in range(gn):
                    if (c, i) in fused:
                        continue
                    t = t_base + i
                    nc.vector.tensor_scalar_add(
                        sts[c][:, i, :], kk_sb[:, c, :], qq_sb[:, c, t : t + 1]
                    )
            for c in range(EC):
                for i in range(gn):
                    if (c, i) not in fused:
                        continue
                    t = t_base + i
                    nc.scalar.activation(
                        out=sts[c][:, i, :], in_=kk_sb[:, c, :],
                        func=AF.Tanh, bias=qq_sb[:, c, t : t + 1],
                    )
            for c in range(EC):
                runs = []
                for i in range(gn):
                    if (c, i) in fused:
                        continue
                    if runs and runs[-1][1] == i:
                        runs[-1][1] = i + 1
                    else:
                        runs.append([i, i + 1])
                for a, b in runs:
                    nc.scalar.activation(
                        out=sts[c][:, a:b, :], in_=sts[c][:, a:b, :], func=AF.Tanh
                    )
            for c in range(EC):
                for i in range(gn):
                    t = t_base + i
                    par = t & 1
                    o = 128 - t if par == 0 else 127 - t
                    nc.tensor.matmul(
                        out=scores_ps,
                        lhsT=sb_wew[:, c, par, o : o + 128],
                        rhs=sts[c][:, i, :],
                        start=False,
                        stop=(g == len(sizes) - 1 and c == EC - 1 and i == gn - 1),
                    )
            t_base += gn

        # ---------------- softmax over j ----------------
        neg_max = work.tile([TBLK, 1], F32, tag="negmax")
        nc.vector.tensor_reduce(
            out=neg_max, in_=scores_ps[0:TBLK, :], axis=mybir.AxisListType.X,
            op=mybir.AluOpType.max, negate=True,
        )
        expt = work.tile([TBLK, jp], F32, tag="expt")
        row_sum = work.tile([TBLK, 1], F32, tag="rowsum")
        nc.scalar.activation(
            out=expt, in_=scores_ps[0:TBLK, :], func=AF.Exp,
            bias=neg_max, scale=1.0, accum_out=row_sum,
        )
        rinv = work.tile([TBLK, 1], F32, tag="rinv")
        nc.vector.reciprocal(rinv, row_sum)
        alphas = work.tile([TBLK, jp], F32, tag="alphas")
        nc.vector.tensor_scalar_mul(alphas, expt, rinv)
        nc.sync.dma_start(out=alp_out[:, :], in_=alphas)

        # ---------------- context = expt @ v, scaled by 1/rowsum ----------
        # Transposes run on the unnormalized exp so they don't wait for the
        # reciprocal; the final copy applies the per-row scale.
        ctx_ps = pkk.tile([TBLK, D], F32, tag="kk")
        for i, (j0, jw) in enumerate(jch):
            tr = pkk.tile([jw, TBLK], F32, tag="kk")
            nc.tensor.transpose(tr, expt[:, j0 : j0 + jw], sb_id)
            alpT = work.tile([jw, TBLK], F32, tag="alpT")
            nc.vector.tensor_copy(alpT, tr)
            nc.tensor.matmul(
                out=ctx_ps, lhsT=alpT, rhs=sb_v[i],
                start=(i == 0), stop=(i == len(jch) - 1),
            )
        ctx_sb = work.tile([TBLK, D], F32, tag="ctxsb")
        nc.vector.tensor_scalar_mul(ctx_sb, ctx_ps, rinv)
        nc.sync.dma_start(out=ctx_out[:, :], in_=ctx_sb)

    # The axon/PJRT execution path serializes the module without calling
    # finalize(), but Bacc's compile passes (reg alloc, wait splitting)
    # must run before lowering.
    nc.finalize()
    return nc


def _prep(k, v, q, mask, Wq, bq, Wk, we):
    """Host-side layout prep: mask compaction, SBUF-image packing, casts."""
    idx = [np.flatnonzero(mask[b, :, 0] != 0) for b in range(BS)]
    ju = [len(ix) for ix in idx]
    jmax = max(max(ju), 1)
    jp = ((jmax + 7) // 8) * 8
    nch, bf_a, fa, bf_b, fb, f32o, ff = _layout(jp)

    def chunked(x):  # [D, n] -> [128, EC*n] p-major image block
        return np.ascontiguousarray(
            x.reshape(EC, 128, -1).transpose(1, 0, 2).reshape(128, -1)
        )

    WkT = chunked(Wk.T.astype(NPBF16))
    WqT = chunked(Wq.T.astype(NPBF16))
    wewin = np.zeros((EC, 2, 128, 256), NPBF16)
    for c in range(EC):
        wewin[c, 0, :, 128] = we[c * 128 : (c + 1) * 128].astype(NPBF16)
        wewin[c, 1, :, 127] = we[c * 128 : (c + 1) * 128].astype(NPBF16)
    wew_img = np.ascontiguousarray(
        wewin.transpose(2, 0, 1, 3).reshape(128, -1)
    )

    imgA_b, imgB_b, imgF_b = [], [], []
    for b in range(BS):
        kt = np.zeros((D, jp), NPBF16)
        kt[:, : ju[b]] = k[b][idx[b]].T.astype(NPBF16)
        a = np.zeros((128, fa), NPBF16)
        a[:, bf_a["Wk"] : bf_a["Wk"] + EC * D] = WkT
        a[:, bf_a["kT"] : bf_a["kT"] + EC * jp] = chunked(kt)
        imgA_b.append(a)

        vv = np.zeros((128, nch * D), np.float32)
        for i in range(nch):
            jw = min(128, jp - i * 128)
            rows = v[b][idx[b]][i * 128 : i * 128 + jw]
            vv[: len(rows), i * D : i * D + D] = rows
        f = np.zeros((128, ff), np.float32)
        f[:, : nch * D] = vv
        f[0, f32o["mrow"] + ju[b] : f32o["mrow"] + jp] = -1e9
        f[:TBLK, f32o["iden"] : f32o["iden"] + TBLK] = np.eye(TBLK)
        imgF_b.append(f)
    qTb = [np.ascontiguousarray(q[b].T).astype(NPBF16) for b in range(BS)]

    in_maps = []
    for core in range(NCORES):
        b = core // (NCORES // BS)
        t0 = (core % (NCORES // BS)) * TBLK
        bimg = np.zeros((128, fb), NPBF16)
        bimg[:, bf_b["Wq"] : bf_b["Wq"] + EC * D] = WqT
        bimg[:, bf_b["qT"] : bf_b["qT"] + EC * TBLK] = chunked(
            qTb[b][:, t0 : t0 + TBLK]
        )
        bimg[:, bf_b["wew"] : bf_b["wew"] + EC * 2 * 256] = wew_img
        bimg[0, bf_b["bq"] : bf_b["bq"] + D] = bq.astype(NPBF16)
        in_maps.append({"imgA": imgA_b[b], "imgB": bimg, "imgF": imgF_b[b]})
    return in_maps, idx, ju, jp


def kernel(**inputs):
    k = np.asarray(inputs["k"], np.float32)
    v = np.asarray(inputs["v"], np.float32)
    q = np.asarray(inputs["q"], np.float32)
    mask = np.asarray(inputs["mask"])
    Wq = np.asarray(inputs["Wq"], np.float32)
    bq = np.asarray(inputs["bq"], np.float32)
    Wk = np.asarray(inputs["Wk"], np.float32)
    we = np.asarray(inputs["we"], np.float32)

    in_maps, idx, ju, jp = _prep(k, v, q, mask, Wq, bq, Wk, we)
    if jp not in _BUILD_CACHE:
        _BUILD_CACHE[jp] = build_nc(jp)
    nc = _BUILD_CACHE[jp]
    res = run_bass_kernel_spmd(nc, in_maps, core_ids=list(range(NCORES))).results

    context = np.zeros((BS, T, D), np.float32)
    alphas = np.zeros((BS, T, J), np.float32)
    for core in range(NCORES):
        b = core // (NCORES // BS)
        t0 = (core % (NCORES // BS)) * TBLK
        context[b, t0 : t0 + TBLK] = res[core]["ctx_out"]
        alphas[b, t0 : t0 + TBLK, idx[b]] = res[core]["alp_out"][:, : ju[b]].T
    # Degenerate all-masked batch (cannot occur for random masks): reference
    # softmax of an all -1e9 row is uniform.
    for b in range(BS):
        if ju[b] == 0:
            alphas[b] = 1.0 / J
            context[b] = alphas[b] @ v[b]
    return context, alphas
